# revision 35
# baseline (speedup 1.0000x reference)
"""Bass/Tile TRN2 kernel for nn_AttentionHead: single-head attention with
q/k/v projections (512->64), key mask, softmax over 4096 keys.

Sharding: 8 cores; core c handles batch c//2, query-half c%2 (2048 queries),
with that batch's full k/v replicated. No collectives.

Host-side prep (layout/dtype/data-movement only):
  - q/k/v pre-transposed to [d, t] bf16 so the device loads contraction-major
    layouts directly: ZERO PE staging transposes, and half the HBM bytes.
  - masked keys are compacted away entirely (gather valid keys, pad to
    T2C=3840 with zero rows and a zero mask column). This is exact: in the
    reference, masked keys hit exp(-1e9 - max) == 0 in fp32, and here the
    zero-padded keys contribute exp(0) * 0 to both numerator and
    denominator. Falls back to the full-4096 kernel if any batch has more
    than T2C valid keys.
  - constants packed into two partition-major tensors (one bf16, one fp32)
    so the whole constant set loads in two large-descriptor DMAs; bv is
    pre-broadcast to [128, 64] and added in the epilogue (out =
    softmax(S) @ V + bv exactly, since masked softmax rows sum to 1).

Per-core dataflow:
  - a short identity-matmul warmup burst keeps the PE busy while the first
    DMAs land, so the HAM clock gate reaches 2.4 GHz before real work.
  - TensorE projections straight from SBUF-resident qT/kT/vT:
    QT/KT [e, t] (bias added on evac, duplicated on partitions 64-127 for
    row-packed scores); V1 [keys, 65] natural layout via matmuls with the
    vT chunk as the stationary operand; column 64 of V1 holds the mask
    (softmax denominator).
  - scores: S^T chunk pairs [t2=128, t1=512] = KT_chunk.T @ QT (contract
    e=64) run concurrently in array row groups 0-63/64-127, landing in
    [128, 1024] fp32 PSUM tiles, double-buffered.
  - ScalarE: exp(0.125 * S^T) per [128, 1024] tile; ACT does nothing else
    (it is the ~60-70us/core floor of this problem).
  - PV: O^T[65, t1] += V1_chunk.T @ expS (row 64 = denominator), emitted
    two groups behind the scores stream so the in-order PE never waits.
  - epilogue: PE transpose [65,128] blocks (double-buffered PSUM),
    reciprocal + scale + bv add on VectorE.
"""

import sys
import types

import numpy as np
import ml_dtypes

import concourse.bass as bass
import concourse.tile as tile
from concourse import bacc, mybir
from concourse.masks import make_identity

B, T1, T2, D, E = 4, 4096, 4096, 512, 64
P = 128
T2C = 3840             # compacted key count (valid keys ~3686 +- 20)
F32 = mybir.dt.float32
BF16 = mybir.dt.bfloat16
BF = ml_dtypes.bfloat16
EXPF = mybir.ActivationFunctionType.Exp


def _install_ntff_hook():
    """Make trace=True usable under axon when antenv.axon_hooks is absent."""
    try:
        import antenv.axon_hooks  # noqa: F401
        return
    except ImportError:
        pass
    try:
        from trn_agent_boot.trn_boot import _ntff_profile_via_ctypes
        hook = _ntff_profile_via_ctypes("/opt/axon/libaxon_pjrt.so")
    except Exception:
        hook = None
    mod = types.ModuleType("antenv.axon_hooks")
    mod.get_axon_ntff_profile_hook = lambda: hook
    mod.set_axon_ntff_profile_hook = lambda h: None
    sys.modules["antenv.axon_hooks"] = mod


def build_body(tc, nc, qT, kT, vT, cpkf, cpkb, out, t1l, t2):
    DC = D // P            # 4 d-chunks
    NT2 = t2 // P          # key chunks
    HW = 512               # t1 half width (one PV accumulator)

    with (
        tc.tile_pool(name="consts", bufs=1) as consts,
        tc.tile_pool(name="persist", bufs=1) as persist,
    ):
        qT_sb = persist.tile([P, DC, t1l], BF16)
        kT_sb = persist.tile([P, DC, t2], BF16)
        vT_sb = persist.tile([P, DC, t2], BF16)
        QT = persist.tile([P, t1l], BF16)
        KT = persist.tile([P, t2], BF16)
        V1 = persist.tile([P, NT2, E + 2], BF16)
        out_sb = persist.tile([P, t1l // P, E], F32)

        cpk_f = consts.tile([P, 98], F32)
        cpk_b = consts.tile([P, 3 * DC * E], BF16)
        ident_f = consts.tile([P, P], F32)
        wup = consts.tile([P, 512], BF16)

        # first query block leads the scalar ring (it gates the whole
        # attention stream); kT/vT stream on the sync ring. first blocks
        # are small so the projection pipeline starts early.
        qrr = qT.rearrange("(c p) t -> p c t", p=P)
        krr = kT.rearrange("(c p) t -> p c t", p=P)
        vrr = vT.rearrange("(c p) t -> p c t", p=P)
        # head blocks split per d-chunk: 128 large descriptors per DMA
        # instead of 512 small ones -> first data lands ~4us earlier, and
        # the projection's accumulation steps chase the chunk arrivals
        for j in range(DC):
            nc.scalar.dma_start(out=qT_sb[:, j, 0:512], in_=qrr[:, j, 0:512])
        nc.scalar.dma_start(out=cpk_b,
                            in_=cpkb.rearrange("(p x) -> p x", p=P))
        nc.scalar.dma_start(out=cpk_f,
                            in_=cpkf.rearrange("(p x) -> p x", p=P))
        nc.scalar.dma_start(out=qT_sb[:, :, 512:1024],
                            in_=qrr[:, :, 512:1024])
        nc.scalar.dma_start(out=qT_sb[:, :, 1024:t1l],
                            in_=qrr[:, :, 1024:t1l])
        for j in range(DC):
            nc.sync.dma_start(out=kT_sb[:, j, 0:512], in_=krr[:, j, 0:512])
        kvblocks = [(512, 512)] + [
            (o, min(1024, t2 - o)) for o in range(1024, t2, 1024)]
        nc.sync.dma_start(out=vT_sb[:, :, 0:512], in_=vrr[:, :, 0:512])
        for o, w in kvblocks:
            nc.sync.dma_start(out=kT_sb[:, :, o:o + w],
                              in_=krr[:, :, o:o + w])
            nc.sync.dma_start(out=vT_sb[:, :, o:o + w],
                              in_=vrr[:, :, o:o + w])

        def wslice(w, j):
            x0 = (w * DC + j) * E
            return cpk_b[:, x0:x0 + E]

        mk = cpk_f[:, 0:NT2]
        bq_s = cpk_f[0:E, 32:33]
        bk_s = cpk_f[0:E, 33:34]
        bv_nat = cpk_f[:, 34:98]

        # PE warmup: dependency-free wide matmuls at >80% duty cycle so the
        # HAM clock gate opens to 2.4 GHz while the input DMAs are in
        # flight (short/narrow matmuls leave the PE below the HAM's
        # busy threshold and it stays at 1.2 GHz).
        nc.gpsimd.memset(wup, 0.0)
        with tc.tile_pool(name="psW", bufs=1, space="PSUM") as psW:
            for i in range(28):
                pw = psW.tile([P, P], F32, tag=f"w{i % 2}", name=f"w_{i}")
                nc.tensor.matmul(pw, wup[:, 0:P], wup[:, 0:P],
                                 start=True, stop=True)

        make_identity(nc, ident_f)
        nc.vector.tensor_copy(out=V1[:, :, E], in_=mk)

        orr = out.rearrange("(n p) e -> p n e", p=P)
        pv_tiles = {}
        pending = []

        with (
            tc.tile_pool(name="psPV", bufs=1, space="PSUM") as psPV,
            tc.tile_pool(name="expp", bufs=6) as expp,
            tc.tile_pool(name="ep", bufs=3) as ep,
        ):
            def emit_pv(item):
                h, c0, nchunk, ex = item
                for u in range(nchunk):
                    c = c0 + u
                    nc.tensor.matmul(
                        pv_tiles[h], V1[:, c, 0:E + 1],
                        ex[:, u * HW:(u + 1) * HW],
                        start=(c == 0), stop=(c == NT2 - 1))

            def scores_exp(pool, c0, nchunk, h, width):
                q0 = h * HW
                ps = pool.tile([P, width * HW], F32, tag="s",
                               name=f"s_{h}_{c0}")
                for u in range(nchunk):
                    c = c0 + u
                    rg = E * (c % 2)
                    nc.tensor.matmul(
                        ps[:, u * HW:(u + 1) * HW],
                        KT[rg:rg + E, c * P:(c + 1) * P],
                        QT[rg:rg + E, q0:q0 + HW], start=True, stop=True,
                        tile_position=(rg, 0))
                ex = expp.tile([P, width * HW], BF16, tag="e",
                               name=f"e_{h}_{c0}")
                nc.scalar.activation(out=ex[:, 0:nchunk * HW],
                                     in_=ps[:, 0:nchunk * HW],
                                     func=EXPF, scale=0.125)
                pending.append((h, c0, nchunk, ex))
                # emit PV in two-group bursts: fewer score<->PV weight/bank
                # switches on the in-order PE
                if len(pending) >= 4:
                    emit_pv(pending.pop(0))
                    emit_pv(pending.pop(0))

            def flush_pv():
                while pending:
                    emit_pv(pending.pop(0))

            def epilogue(h, psO):
                pvt = pv_tiles.pop(h)
                q0 = h * HW
                n0, n1 = q0 // P, (q0 + HW) // P
                ov = ep.tile([E + 1, HW], F32, tag="ov", name=f"ov_{h}")
                nc.vector.tensor_copy(out=ov, in_=pvt)
                for j in range(HW // P):
                    po = psO.tile([P, E + 1], F32, tag=f"o{j % 2}",
                                  name=f"o_{h}_{j}")
                    nc.tensor.transpose(
                        po, ov[:, j * P:(j + 1) * P],
                        ident_f[0:E + 1, 0:E + 1])
                    rec = ep.tile([P, 1], F32, tag="rec", name=f"rec_{h}_{j}")
                    nc.vector.reciprocal(rec, po[:, E:E + 1])
                    nb = (q0 + j * P) // P
                    nc.vector.scalar_tensor_tensor(
                        out_sb[:, nb, :], po[:, 0:E], rec, bv_nat,
                        mybir.AluOpType.mult, mybir.AluOpType.add)
                nc.sync.dma_start(out=orr[:, n0:n1, :],
                                  in_=out_sb[:, n0:n1, :])

            # ---------------- stage 1: project + first two halves --------
            with (
                tc.tile_pool(name="psS1", bufs=2, space="PSUM") as psS1,
                tc.tile_pool(name="psP", bufs=1, space="PSUM") as psP,
                tc.tile_pool(name="psV", bufs=1, space="PSUM") as psV,
            ):
                def proj_qk(src_sb, w, b_s, dst, o, wid):
                    sl = slice(o, o + wid)
                    ps = psP.tile([E, 512], F32, tag="pp",
                                  name=f"p_{dst.tensor.name}_{o}")
                    for j in range(DC):
                        nc.tensor.matmul(ps[:, 0:wid], wslice(w, j),
                                         src_sb[:, j, sl],
                                         start=(j == 0), stop=(j == DC - 1))
                    nc.vector.tensor_scalar_add(dst[0:E, sl], ps[:, 0:wid],
                                                b_s)
                    nc.vector.tensor_copy(out=dst[E:2 * E, sl],
                                          in_=dst[0:E, sl])

                def proj_v(c0, nch):
                    ps = psV.tile([P, 4, E], F32, tag="pv", name=f"v_{c0}")
                    for ci in range(nch):
                        c = c0 + ci
                        for j in range(DC):
                            nc.tensor.matmul(
                                ps[:, ci, :], vT_sb[:, j, c * P:(c + 1) * P],
                                wslice(2, j), start=(j == 0),
                                stop=(j == DC - 1))
                    nc.vector.tensor_copy(
                        out=V1[:, c0:c0 + nch, 0:E], in_=ps[:, 0:nch, :])

                # k/v head of the pipeline first: it is ready before qT
                proj_qk(kT_sb, 1, bk_s, KT, 0, 512)
                proj_v(0, 4)
                proj_qk(qT_sb, 0, bq_s, QT, 0, 512)
                proj_qk(qT_sb, 0, bq_s, QT, 512, 512)

                for h in (0, 1):
                    pv_tiles[h] = psPV.tile([E + 1, HW], F32,
                                            tag=f"pv{h % 2}", name=f"pv_{h}")
                for sub in range((t2 + 511) // 512):
                    o = sub * 512
                    wid = min(512, t2 - o)
                    if sub > 0:
                        proj_qk(kT_sb, 1, bk_s, KT, o, wid)
                        proj_v(o // P, wid // P)
                    if sub == 2:
                        proj_qk(qT_sb, 0, bq_s, QT, 1024, 512)
                        proj_qk(qT_sb, 0, bq_s, QT, 1536, 512)
                    for cp in range(wid // 256):
                        for h in (0, 1):
                            scores_exp(psS1, o // P + cp * 2, 2, h, 2)

            # ---------------- stage 2: remaining halves + epilogues ------
            with (
                tc.tile_pool(name="psS2", bufs=2, space="PSUM") as psS2,
                tc.tile_pool(name="psO", bufs=1, space="PSUM") as psO,
            ):
                flush_pv()
                stream_epi = [0, 1]
                epilogue(stream_epi.pop(0), psO)
                for h in (2, 3):
                    pv_tiles[h] = psPV.tile([E + 1, HW], F32,
                                            tag=f"pv{h % 2}", name=f"pv_{h}")
                    for g in range(NT2 // 2):
                        scores_exp(psS2, g * 2, 2, h, 2)
                        if g == 2 and stream_epi:
                            epilogue(stream_epi.pop(0), psO)
                    flush_pv()
                    epilogue(h, psO)


def build_nc(t1l=T1 // 2, t2=T2):
    nc = bacc.Bacc()
    qT = nc.declare_dram_parameter("qT", [D, t1l], BF16, isOutput=False)
    kT = nc.declare_dram_parameter("kT", [D, t2], BF16, isOutput=False)
    vT = nc.declare_dram_parameter("vT", [D, t2], BF16, isOutput=False)
    cpkf = nc.declare_dram_parameter("cpkf", [P * 98], F32, isOutput=False)
    cpkb = nc.declare_dram_parameter("cpkb", [3 * D * E], BF16,
                                     isOutput=False)
    out = nc.declare_dram_parameter("out", [t1l, E], F32, isOutput=True)
    with tile.TileContext(nc) as tc:
        build_body(tc, nc, qT[:], kT[:], vT[:], cpkf[:], cpkb[:], out[:],
                   t1l, t2)
    nc.compile()
    return nc


_NC_CACHE = {}


def _get_nc(t2):
    if t2 not in _NC_CACHE:
        _NC_CACHE[t2] = build_nc(t2=t2)
    return _NC_CACHE[t2]


def make_in_maps(q, k, v, mask, Wq, bq, Wk, bk, Wv, bv):
    t1l = T1 // 2
    q = np.asarray(q, np.float32)
    k = np.asarray(k, np.float32)
    v = np.asarray(v, np.float32)
    mask = np.asarray(mask, np.float32)
    qbf = q.astype(BF)

    # compact away masked keys (exact; see module docstring)
    valid = [np.nonzero(mask[b, 0] != 0.0)[0] for b in range(B)]
    if max(len(ix) for ix in valid) <= T2C:
        t2 = T2C
    else:
        t2 = T2
        valid = [np.arange(T2) for _ in range(B)]

    # packed bf16 constants, partition-major: row p = [w, chunk, e] with
    # value W_w[chunk*128 + p, e]
    ws = np.stack([np.asarray(W, np.float32).astype(BF)
                   for W in (Wq, Wk, Wv)])          # [3, 512, 64]
    cpkb = np.ascontiguousarray(
        ws.reshape(3, 4, P, E).transpose(2, 0, 1, 3).reshape(P, -1)
    ).reshape(-1)

    in_maps = []
    for c in range(8):
        b, h = divmod(c, 2)
        ix = valid[b]
        nv = len(ix)
        kc = np.zeros((t2, D), BF)
        kc[:nv] = k[b, ix].astype(BF)
        vc = np.zeros((t2, D), BF)
        vc[:nv] = (v[b, ix] * mask[b, 0, ix, None]).astype(BF)
        mc = np.zeros(t2, np.float32)
        mc[:nv] = mask[b, 0, ix]
        # packed fp32 constants: mask cols | bq | bk | bv broadcast
        cpkf = np.zeros((P, 98), np.float32)
        cpkf[:, 0:t2 // P] = mc.reshape(t2 // P, P).T
        cpkf[0:E, 32] = np.asarray(bq, np.float32)
        cpkf[0:E, 33] = np.asarray(bk, np.float32)
        cpkf[:, 34:98] = np.asarray(bv, np.float32)[None, :]
        in_maps.append({
            "qT": np.ascontiguousarray(qbf[b, h * t1l:(h + 1) * t1l].T),
            "kT": np.ascontiguousarray(kc.T),
            "vT": np.ascontiguousarray(vc.T),
            "cpkf": np.ascontiguousarray(cpkf.reshape(-1)),
            "cpkb": cpkb,
        })
    return in_maps, t2


def assemble_out(results):
    t1l = T1 // 2
    out = np.empty((B, T1, E), np.float32)
    for c in range(8):
        b, h = divmod(c, 2)
        out[b, h * t1l:(h + 1) * t1l] = results[c]["out"]
    return out


def run(inputs, trace=False):
    from concourse.bass_utils import run_bass_kernel_spmd
    _install_ntff_hook()
    in_maps, t2 = make_in_maps(**inputs)
    nc = _get_nc(t2)
    res = run_bass_kernel_spmd(nc, in_maps, list(range(8)), trace=trace)
    return assemble_out(res.results), res


def kernel(q, k, v, mask, Wq, bq, Wk, bk, Wv, bv):
    out, _ = run(dict(q=q, k=k, v=v, mask=mask, Wq=Wq, bq=bq, Wk=Wk, bk=bk,
                      Wv=Wv, bv=bv))
    return out


# revision 37
# speedup vs baseline: 1.1219x; 1.1219x over previous
"""Bass/Tile TRN2 kernel for nn_AttentionHead: single-head attention with
q/k/v projections (512->64), key mask, softmax over 4096 keys.

Sharding: 8 cores; core c handles batch c//2, query-half c%2 (2048 queries),
with that batch's full k/v replicated. No collectives.

Host-side prep (layout/dtype/data-movement only):
  - q/k/v pre-transposed to [d, t] bf16 so the device loads contraction-major
    layouts directly: ZERO PE staging transposes, and half the HBM bytes.
  - masked keys are compacted away entirely (gather valid keys, pad to
    T2C=3840 with zero rows and a zero mask column). This is exact: in the
    reference, masked keys hit exp(-1e9 - max) == 0 in fp32, and here the
    zero-padded keys contribute exp(0) * 0 to both numerator and
    denominator. Falls back to the full-4096 kernel if any batch has more
    than T2C valid keys.
  - constants packed into two partition-major tensors (one bf16, one fp32)
    so the whole constant set loads in two large-descriptor DMAs; bv is
    pre-broadcast to [128, 64] and added in the epilogue (out =
    softmax(S) @ V + bv exactly, since masked softmax rows sum to 1).

Per-core dataflow:
  - a short identity-matmul warmup burst keeps the PE busy while the first
    DMAs land, so the HAM clock gate reaches 2.4 GHz before real work.
  - TensorE projections straight from SBUF-resident qT/kT/vT:
    QT/KT [e, t] (bias added on evac, duplicated on partitions 64-127 for
    row-packed scores); V1 [keys, 65] natural layout via matmuls with the
    vT chunk as the stationary operand; column 64 of V1 holds the mask
    (softmax denominator).
  - scores: S^T chunk pairs [t2=128, t1=512] = KT_chunk.T @ QT (contract
    e=64) run concurrently in array row groups 0-63/64-127, landing in
    [128, 1024] fp32 PSUM tiles, double-buffered.
  - ScalarE: exp(0.125 * S^T) per [128, 1024] tile; ACT does nothing else
    (it is the ~60-70us/core floor of this problem).
  - PV: O^T[65, t1] += V1_chunk.T @ expS (row 64 = denominator), emitted
    two groups behind the scores stream so the in-order PE never waits.
  - epilogue: PE transpose [65,128] blocks (double-buffered PSUM),
    reciprocal + scale + bv add on VectorE.
"""

import sys
import types

import numpy as np
import ml_dtypes

import concourse.bass as bass
import concourse.tile as tile
from concourse import bacc, mybir
from concourse.masks import make_identity

B, T1, T2, D, E = 4, 4096, 4096, 512, 64
P = 128
T2C = 3840             # compacted key count (valid keys ~3686 +- 20)
F32 = mybir.dt.float32
BF16 = mybir.dt.bfloat16
BF = ml_dtypes.bfloat16
EXPF = mybir.ActivationFunctionType.Exp


def _install_ntff_hook():
    """Make trace=True usable under axon when antenv.axon_hooks is absent."""
    try:
        import antenv.axon_hooks  # noqa: F401
        return
    except ImportError:
        pass
    try:
        from trn_agent_boot.trn_boot import _ntff_profile_via_ctypes
        hook = _ntff_profile_via_ctypes("/opt/axon/libaxon_pjrt.so")
    except Exception:
        hook = None
    mod = types.ModuleType("antenv.axon_hooks")
    mod.get_axon_ntff_profile_hook = lambda: hook
    mod.set_axon_ntff_profile_hook = lambda h: None
    sys.modules["antenv.axon_hooks"] = mod


def build_body(tc, nc, qT, kT, vT, cpkf, cpkb, out, t1l, t2):
    DC = D // P            # 4 d-chunks
    NT2 = t2 // P          # key chunks
    HW = 512               # t1 half width (one PV accumulator)

    with (
        tc.tile_pool(name="consts", bufs=1) as consts,
        tc.tile_pool(name="persist", bufs=1) as persist,
    ):
        qT_sb = persist.tile([P, DC, t1l], BF16)
        kT_sb = persist.tile([P, DC, t2], BF16)
        vT_sb = persist.tile([P, DC, t2], BF16)
        QT = persist.tile([P, t1l], BF16)
        KT = persist.tile([P, t2], BF16)
        V1 = persist.tile([P, NT2, E + 2], BF16)
        out_sb = persist.tile([P, t1l // P, E], F32)

        cpk_f = consts.tile([P, 98], F32)
        cpk_b = consts.tile([P, 3 * DC * E], BF16)
        ident_f = consts.tile([P, P], F32)
        wup = consts.tile([P, 512], BF16)

        # first query block leads the scalar ring (it gates the whole
        # attention stream); kT/vT stream on the sync ring. first blocks
        # are small so the projection pipeline starts early.
        qrr = qT.rearrange("(c p) t -> p c t", p=P)
        krr = kT.rearrange("(c p) t -> p c t", p=P)
        vrr = vT.rearrange("(c p) t -> p c t", p=P)
        nc.scalar.dma_start(out=qT_sb[:, :, 0:512], in_=qrr[:, :, 0:512])
        nc.scalar.dma_start(out=cpk_b,
                            in_=cpkb.rearrange("(p x) -> p x", p=P))
        nc.scalar.dma_start(out=cpk_f,
                            in_=cpkf.rearrange("(p x) -> p x", p=P))
        nc.scalar.dma_start(out=qT_sb[:, :, 512:1024],
                            in_=qrr[:, :, 512:1024])
        nc.scalar.dma_start(out=qT_sb[:, :, 1024:t1l],
                            in_=qrr[:, :, 1024:t1l])
        kvblocks = [(0, 512), (512, 512)] + [
            (o, min(1024, t2 - o)) for o in range(1024, t2, 1024)]
        for o, w in kvblocks:
            nc.sync.dma_start(out=kT_sb[:, :, o:o + w],
                              in_=krr[:, :, o:o + w])
            nc.sync.dma_start(out=vT_sb[:, :, o:o + w],
                              in_=vrr[:, :, o:o + w])

        def wslice(w, j):
            x0 = (w * DC + j) * E
            return cpk_b[:, x0:x0 + E]

        mk = cpk_f[:, 0:NT2]
        bq_s = cpk_f[0:E, 32:33]
        bk_s = cpk_f[0:E, 33:34]
        bv_nat = cpk_f[:, 34:98]

        # PE warmup: dependency-free wide matmuls at >80% duty cycle so the
        # HAM clock gate opens to 2.4 GHz while the input DMAs are in
        # flight (short/narrow matmuls leave the PE below the HAM's
        # busy threshold and it stays at 1.2 GHz).
        nc.gpsimd.memset(wup, 0.0)
        with tc.tile_pool(name="psW", bufs=1, space="PSUM") as psW:
            for i in range(28):
                pw = psW.tile([P, P], F32, tag=f"w{i % 2}", name=f"w_{i}")
                nc.tensor.matmul(pw, wup[:, 0:P], wup[:, 0:P],
                                 start=True, stop=True)

        make_identity(nc, ident_f)
        nc.vector.tensor_copy(out=V1[:, :, E], in_=mk)

        orr = out.rearrange("(n p) e -> p n e", p=P)
        pv_tiles = {}
        pending = []

        with (
            tc.tile_pool(name="psPV", bufs=1, space="PSUM") as psPV,
            tc.tile_pool(name="expp", bufs=6) as expp,
            tc.tile_pool(name="ep", bufs=3) as ep,
        ):
            def emit_pv(item):
                h, c0, nchunk, ex = item
                for u in range(nchunk):
                    c = c0 + u
                    nc.tensor.matmul(
                        pv_tiles[h], V1[:, c, 0:E + 1],
                        ex[:, u * HW:(u + 1) * HW],
                        start=(c == 0), stop=(c == NT2 - 1))

            def scores_exp(pool, c0, nchunk, h, width):
                q0 = h * HW
                ps = pool.tile([P, width * HW], F32, tag="s",
                               name=f"s_{h}_{c0}")
                for u in range(nchunk):
                    c = c0 + u
                    rg = E * (c % 2)
                    nc.tensor.matmul(
                        ps[:, u * HW:(u + 1) * HW],
                        KT[rg:rg + E, c * P:(c + 1) * P],
                        QT[rg:rg + E, q0:q0 + HW], start=True, stop=True,
                        tile_position=(rg, 0))
                ex = expp.tile([P, width * HW], BF16, tag="e",
                               name=f"e_{h}_{c0}")
                nc.scalar.activation(out=ex[:, 0:nchunk * HW],
                                     in_=ps[:, 0:nchunk * HW],
                                     func=EXPF, scale=0.125)
                pending.append((h, c0, nchunk, ex))
                # emit PV in two-group bursts: fewer score<->PV weight/bank
                # switches on the in-order PE
                if len(pending) >= 4:
                    emit_pv(pending.pop(0))
                    emit_pv(pending.pop(0))

            def flush_pv():
                while pending:
                    emit_pv(pending.pop(0))

            def epilogue(h, psO):
                pvt = pv_tiles.pop(h)
                q0 = h * HW
                n0, n1 = q0 // P, (q0 + HW) // P
                ov = ep.tile([E + 1, HW], F32, tag="ov", name=f"ov_{h}")
                nc.vector.tensor_copy(out=ov, in_=pvt)
                for j in range(HW // P):
                    po = psO.tile([P, E + 1], F32, tag=f"o{j % 2}",
                                  name=f"o_{h}_{j}")
                    nc.tensor.transpose(
                        po, ov[:, j * P:(j + 1) * P],
                        ident_f[0:E + 1, 0:E + 1])
                    rec = ep.tile([P, 1], F32, tag="rec", name=f"rec_{h}_{j}")
                    nc.vector.reciprocal(rec, po[:, E:E + 1])
                    nb = (q0 + j * P) // P
                    nc.vector.scalar_tensor_tensor(
                        out_sb[:, nb, :], po[:, 0:E], rec, bv_nat,
                        mybir.AluOpType.mult, mybir.AluOpType.add)
                nc.sync.dma_start(out=orr[:, n0:n1, :],
                                  in_=out_sb[:, n0:n1, :])

            # ---------------- stage 1: project + first two halves --------
            with (
                tc.tile_pool(name="psS1", bufs=2, space="PSUM") as psS1,
                tc.tile_pool(name="psP", bufs=1, space="PSUM") as psP,
                tc.tile_pool(name="psV", bufs=1, space="PSUM") as psV,
            ):
                def proj_qk(src_sb, w, b_s, dst, o, wid):
                    sl = slice(o, o + wid)
                    ps = psP.tile([E, 512], F32, tag="pp",
                                  name=f"p_{dst.tensor.name}_{o}")
                    for j in range(DC):
                        nc.tensor.matmul(ps[:, 0:wid], wslice(w, j),
                                         src_sb[:, j, sl],
                                         start=(j == 0), stop=(j == DC - 1))
                    nc.vector.tensor_scalar_add(dst[0:E, sl], ps[:, 0:wid],
                                                b_s)
                    nc.vector.tensor_copy(out=dst[E:2 * E, sl],
                                          in_=dst[0:E, sl])

                def proj_v(c0, nch):
                    ps = psV.tile([P, 4, E], F32, tag="pv", name=f"v_{c0}")
                    for ci in range(nch):
                        c = c0 + ci
                        for j in range(DC):
                            nc.tensor.matmul(
                                ps[:, ci, :], vT_sb[:, j, c * P:(c + 1) * P],
                                wslice(2, j), start=(j == 0),
                                stop=(j == DC - 1))
                    nc.vector.tensor_copy(
                        out=V1[:, c0:c0 + nch, 0:E], in_=ps[:, 0:nch, :])

                # k/v head of the pipeline first: it is ready before qT
                proj_qk(kT_sb, 1, bk_s, KT, 0, 512)
                proj_v(0, 4)
                proj_qk(qT_sb, 0, bq_s, QT, 0, 512)
                proj_qk(qT_sb, 0, bq_s, QT, 512, 512)

                for h in (0, 1):
                    pv_tiles[h] = psPV.tile([E + 1, HW], F32,
                                            tag=f"pv{h % 2}", name=f"pv_{h}")
                for sub in range((t2 + 511) // 512):
                    o = sub * 512
                    wid = min(512, t2 - o)
                    if sub > 0:
                        proj_qk(kT_sb, 1, bk_s, KT, o, wid)
                        proj_v(o // P, wid // P)
                    if sub == 2:
                        proj_qk(qT_sb, 0, bq_s, QT, 1024, 512)
                        proj_qk(qT_sb, 0, bq_s, QT, 1536, 512)
                    for cp in range(wid // 256):
                        for h in (0, 1):
                            scores_exp(psS1, o // P + cp * 2, 2, h, 2)

            # ---------------- stage 2: remaining halves + epilogues ------
            with (
                tc.tile_pool(name="psS2", bufs=2, space="PSUM") as psS2,
                tc.tile_pool(name="psO", bufs=1, space="PSUM") as psO,
            ):
                flush_pv()
                stream_epi = [0, 1]
                epilogue(stream_epi.pop(0), psO)
                for h in (2, 3):
                    pv_tiles[h] = psPV.tile([E + 1, HW], F32,
                                            tag=f"pv{h % 2}", name=f"pv_{h}")
                    for g in range(NT2 // 2):
                        scores_exp(psS2, g * 2, 2, h, 2)
                        if g == 2 and stream_epi:
                            epilogue(stream_epi.pop(0), psO)
                    flush_pv()
                    epilogue(h, psO)


def build_nc(t1l=T1 // 2, t2=T2):
    nc = bacc.Bacc()
    qT = nc.declare_dram_parameter("qT", [D, t1l], BF16, isOutput=False)
    kT = nc.declare_dram_parameter("kT", [D, t2], BF16, isOutput=False)
    vT = nc.declare_dram_parameter("vT", [D, t2], BF16, isOutput=False)
    cpkf = nc.declare_dram_parameter("cpkf", [P * 98], F32, isOutput=False)
    cpkb = nc.declare_dram_parameter("cpkb", [3 * D * E], BF16,
                                     isOutput=False)
    out = nc.declare_dram_parameter("out", [t1l, E], F32, isOutput=True)
    with tile.TileContext(nc) as tc:
        build_body(tc, nc, qT[:], kT[:], vT[:], cpkf[:], cpkb[:], out[:],
                   t1l, t2)
    nc.compile()
    return nc


_NC_CACHE = {}


def _get_nc(t2):
    if t2 not in _NC_CACHE:
        _NC_CACHE[t2] = build_nc(t2=t2)
    return _NC_CACHE[t2]


def make_in_maps(q, k, v, mask, Wq, bq, Wk, bk, Wv, bv):
    t1l = T1 // 2
    q = np.asarray(q, np.float32)
    k = np.asarray(k, np.float32)
    v = np.asarray(v, np.float32)
    mask = np.asarray(mask, np.float32)
    qbf = q.astype(BF)

    # compact away masked keys (exact; see module docstring)
    valid = [np.nonzero(mask[b, 0] != 0.0)[0] for b in range(B)]
    if max(len(ix) for ix in valid) <= T2C:
        t2 = T2C
    else:
        t2 = T2
        valid = [np.arange(T2) for _ in range(B)]

    # packed bf16 constants, partition-major: row p = [w, chunk, e] with
    # value W_w[chunk*128 + p, e]
    ws = np.stack([np.asarray(W, np.float32).astype(BF)
                   for W in (Wq, Wk, Wv)])          # [3, 512, 64]
    cpkb = np.ascontiguousarray(
        ws.reshape(3, 4, P, E).transpose(2, 0, 1, 3).reshape(P, -1)
    ).reshape(-1)

    in_maps = []
    for c in range(8):
        b, h = divmod(c, 2)
        ix = valid[b]
        nv = len(ix)
        kc = np.zeros((t2, D), BF)
        kc[:nv] = k[b, ix].astype(BF)
        vc = np.zeros((t2, D), BF)
        vc[:nv] = (v[b, ix] * mask[b, 0, ix, None]).astype(BF)
        mc = np.zeros(t2, np.float32)
        mc[:nv] = mask[b, 0, ix]
        # packed fp32 constants: mask cols | bq | bk | bv broadcast
        cpkf = np.zeros((P, 98), np.float32)
        cpkf[:, 0:t2 // P] = mc.reshape(t2 // P, P).T
        cpkf[0:E, 32] = np.asarray(bq, np.float32)
        cpkf[0:E, 33] = np.asarray(bk, np.float32)
        cpkf[:, 34:98] = np.asarray(bv, np.float32)[None, :]
        in_maps.append({
            "qT": np.ascontiguousarray(qbf[b, h * t1l:(h + 1) * t1l].T),
            "kT": np.ascontiguousarray(kc.T),
            "vT": np.ascontiguousarray(vc.T),
            "cpkf": np.ascontiguousarray(cpkf.reshape(-1)),
            "cpkb": cpkb,
        })
    return in_maps, t2


def assemble_out(results):
    t1l = T1 // 2
    out = np.empty((B, T1, E), np.float32)
    for c in range(8):
        b, h = divmod(c, 2)
        out[b, h * t1l:(h + 1) * t1l] = results[c]["out"]
    return out


def run(inputs, trace=False):
    from concourse.bass_utils import run_bass_kernel_spmd
    _install_ntff_hook()
    in_maps, t2 = make_in_maps(**inputs)
    nc = _get_nc(t2)
    res = run_bass_kernel_spmd(nc, in_maps, list(range(8)), trace=trace)
    return assemble_out(res.results), res


def kernel(q, k, v, mask, Wq, bq, Wk, bk, Wv, bv):
    out, _ = run(dict(q=q, k=k, v=v, mask=mask, Wq=Wq, bq=bq, Wk=Wk, bk=bk,
                      Wv=Wv, bv=bv))
    return out


# revision 40
# speedup vs baseline: 1.1497x; 1.0248x over previous
"""Bass/Tile TRN2 kernel for nn_AttentionHead: single-head attention with
q/k/v projections (512->64), key mask, softmax over 4096 keys.

Sharding: 8 cores; core c handles batch c//2, query-half c%2 (2048 queries),
with that batch's full k/v replicated. No collectives.

Host-side prep (layout/dtype/data-movement only):
  - q/k/v pre-transposed to [d, t] bf16 so the device loads contraction-major
    layouts directly: ZERO PE staging transposes, and half the HBM bytes.
  - masked keys are compacted away entirely (gather valid keys, pad to
    T2C=3840 with zero rows and a zero mask column). This is exact: in the
    reference, masked keys hit exp(-1e9 - max) == 0 in fp32, and here the
    zero-padded keys contribute exp(0) * 0 to both numerator and
    denominator. Falls back to the full-4096 kernel if any batch has more
    than T2C valid keys.
  - constants packed into two partition-major tensors (one bf16, one fp32)
    so the whole constant set loads in two large-descriptor DMAs; bv is
    pre-broadcast to [128, 64] and added in the epilogue (out =
    softmax(S) @ V + bv exactly, since masked softmax rows sum to 1).

Per-core dataflow:
  - a short identity-matmul warmup burst keeps the PE busy while the first
    DMAs land, so the HAM clock gate reaches 2.4 GHz before real work.
  - TensorE projections straight from SBUF-resident qT/kT/vT:
    QT/KT [e, t] (bias added on evac, duplicated on partitions 64-127 for
    row-packed scores); V1 [keys, 65] natural layout via matmuls with the
    vT chunk as the stationary operand; column 64 of V1 holds the mask
    (softmax denominator).
  - scores: S^T chunk pairs [t2=128, t1=512] = KT_chunk.T @ QT (contract
    e=64) run concurrently in array row groups 0-63/64-127, landing in
    [128, 1024] fp32 PSUM tiles, double-buffered.
  - ScalarE: exp(0.125 * S^T) per [128, 1024] tile; ACT does nothing else
    (it is the ~60-70us/core floor of this problem).
  - PV: O^T[65, t1] += V1_chunk.T @ expS (row 64 = denominator), emitted
    two groups behind the scores stream so the in-order PE never waits.
  - epilogue: PE transpose [65,128] blocks (double-buffered PSUM),
    reciprocal + scale + bv add on VectorE.
"""

import sys
import types

import numpy as np
import ml_dtypes

import concourse.bass as bass
import concourse.tile as tile
from concourse import bacc, mybir
from concourse.masks import make_identity

B, T1, T2, D, E = 4, 4096, 4096, 512, 64
P = 128
T2C = 3840             # compacted key count (valid keys ~3686 +- 20)
F32 = mybir.dt.float32
BF16 = mybir.dt.bfloat16
BF = ml_dtypes.bfloat16
EXPF = mybir.ActivationFunctionType.Exp


def _install_ntff_hook():
    """Make trace=True usable under axon when antenv.axon_hooks is absent."""
    try:
        import antenv.axon_hooks  # noqa: F401
        return
    except ImportError:
        pass
    try:
        from trn_agent_boot.trn_boot import _ntff_profile_via_ctypes
        hook = _ntff_profile_via_ctypes("/opt/axon/libaxon_pjrt.so")
    except Exception:
        hook = None
    mod = types.ModuleType("antenv.axon_hooks")
    mod.get_axon_ntff_profile_hook = lambda: hook
    mod.set_axon_ntff_profile_hook = lambda h: None
    sys.modules["antenv.axon_hooks"] = mod


def build_body(tc, nc, qT, kT, vT, cpkf, cpkb, out, t1l, t2):
    DC = D // P            # 4 d-chunks
    NT2 = t2 // P          # key chunks
    HW = 512               # t1 half width (one PV accumulator)

    with (
        tc.tile_pool(name="consts", bufs=1) as consts,
        tc.tile_pool(name="persist", bufs=1) as persist,
    ):
        qT_sb = persist.tile([P, DC, t1l], BF16)
        kT_sb = persist.tile([P, DC, t2], BF16)
        vT_sb = persist.tile([P, DC, t2], BF16)
        QT = persist.tile([P, t1l], BF16)
        KT = persist.tile([P, t2], BF16)
        V1 = persist.tile([P, NT2, E + 2], BF16)
        out_sb = persist.tile([P, t1l // P, E], F32)

        cpk_f = consts.tile([P, 98], F32)
        cpk_b = consts.tile([P, 3 * DC * E], BF16)
        ident_f = consts.tile([P, P], F32)
        wup = consts.tile([P, 512], BF16)

        # first query block leads the scalar ring (it gates the whole
        # attention stream); kT/vT stream on the sync ring. first blocks
        # are small so the projection pipeline starts early.
        qrr = qT.rearrange("(c p) t -> p c t", p=P)
        krr = kT.rearrange("(c p) t -> p c t", p=P)
        vrr = vT.rearrange("(c p) t -> p c t", p=P)
        nc.scalar.dma_start(out=qT_sb[:, :, 0:512], in_=qrr[:, :, 0:512])
        nc.scalar.dma_start(out=cpk_b,
                            in_=cpkb.rearrange("(p x) -> p x", p=P))
        nc.scalar.dma_start(out=cpk_f,
                            in_=cpkf.rearrange("(p x) -> p x", p=P))
        nc.scalar.dma_start(out=qT_sb[:, :, 512:1024],
                            in_=qrr[:, :, 512:1024])
        nc.scalar.dma_start(out=qT_sb[:, :, 1024:t1l],
                            in_=qrr[:, :, 1024:t1l])
        kvblocks = [(0, 512), (512, 512)] + [
            (o, min(1024, t2 - o)) for o in range(1024, t2, 1024)]
        for o, w in kvblocks:
            nc.sync.dma_start(out=kT_sb[:, :, o:o + w],
                              in_=krr[:, :, o:o + w])
            nc.sync.dma_start(out=vT_sb[:, :, o:o + w],
                              in_=vrr[:, :, o:o + w])

        def wslice(w, j):
            x0 = (w * DC + j) * E
            return cpk_b[:, x0:x0 + E]

        mk = cpk_f[:, 0:NT2]
        bq_s = cpk_f[0:E, 32:33]
        bk_s = cpk_f[0:E, 33:34]
        bv_nat = cpk_f[:, 34:98]

        # PE warmup: dependency-free wide matmuls at >80% duty cycle so the
        # HAM clock gate opens to 2.4 GHz while the input DMAs are in
        # flight (short/narrow matmuls leave the PE below the HAM's
        # busy threshold and it stays at 1.2 GHz).
        nc.gpsimd.memset(wup, 0.0)
        with tc.tile_pool(name="psW", bufs=1, space="PSUM") as psW:
            for i in range(28):
                pw = psW.tile([P, P], F32, tag=f"w{i % 2}", name=f"w_{i}")
                nc.tensor.matmul(pw, wup[:, 0:P], wup[:, 0:P],
                                 start=True, stop=True)

        make_identity(nc, ident_f)
        nc.vector.tensor_copy(out=V1[:, :, E], in_=mk)

        orr = out.rearrange("(n p) e -> p n e", p=P)
        pv_tiles = {}
        pending = []

        with (
            tc.tile_pool(name="psPV", bufs=1, space="PSUM") as psPV,
            tc.tile_pool(name="expp", bufs=6) as expp,
            tc.tile_pool(name="ep", bufs=3) as ep,
        ):
            def emit_pv(item):
                h, c0, nchunk, ex = item
                for u in range(nchunk):
                    c = c0 + u
                    nc.tensor.matmul(
                        pv_tiles[h], V1[:, c, 0:E + 1],
                        ex[:, u * HW:(u + 1) * HW],
                        start=(c == 0), stop=(c == NT2 - 1))

            def scores_exp(pool, c0, nchunk, h, width):
                q0 = h * HW
                ps = pool.tile([P, width * HW], F32, tag="s",
                               name=f"s_{h}_{c0}")
                for u in range(nchunk):
                    c = c0 + u
                    rg = E * (c % 2)
                    nc.tensor.matmul(
                        ps[:, u * HW:(u + 1) * HW],
                        KT[rg:rg + E, c * P:(c + 1) * P],
                        QT[rg:rg + E, q0:q0 + HW], start=True, stop=True,
                        tile_position=(rg, 0))
                ex = expp.tile([P, width * HW], BF16, tag="e",
                               name=f"e_{h}_{c0}")
                nc.scalar.activation(out=ex[:, 0:nchunk * HW],
                                     in_=ps[:, 0:nchunk * HW],
                                     func=EXPF, scale=0.125)
                pending.append((h, c0, nchunk, ex))
                # emit PV in two-group bursts: fewer score<->PV weight/bank
                # switches on the in-order PE
                if len(pending) >= 4:
                    emit_pv(pending.pop(0))
                    emit_pv(pending.pop(0))

            def flush_pv():
                while pending:
                    emit_pv(pending.pop(0))

            def epilogue(h, psO):
                pvt = pv_tiles.pop(h)
                q0 = h * HW
                n0, n1 = q0 // P, (q0 + HW) // P
                ov = ep.tile([E + 1, HW], F32, tag="ov", name=f"ov_{h}")
                nc.vector.tensor_copy(out=ov, in_=pvt)
                for j in range(HW // P):
                    po = psO.tile([P, E + 1], F32, tag=f"o{j % 2}",
                                  name=f"o_{h}_{j}")
                    nc.tensor.transpose(
                        po, ov[:, j * P:(j + 1) * P],
                        ident_f[0:E + 1, 0:E + 1])
                    rec = ep.tile([P, 1], F32, tag="rec", name=f"rec_{h}_{j}")
                    nc.vector.reciprocal(rec, po[:, E:E + 1])
                    nb = (q0 + j * P) // P
                    nc.vector.scalar_tensor_tensor(
                        out_sb[:, nb, :], po[:, 0:E], rec, bv_nat,
                        mybir.AluOpType.mult, mybir.AluOpType.add)
                nc.sync.dma_start(out=orr[:, n0:n1, :],
                                  in_=out_sb[:, n0:n1, :])

            # ---------------- stage 1: project + first two halves --------
            with (
                tc.tile_pool(name="psS1", bufs=2, space="PSUM") as psS1,
                tc.tile_pool(name="psP", bufs=1, space="PSUM") as psP,
                tc.tile_pool(name="psV", bufs=1, space="PSUM") as psV,
            ):
                def proj_qk(src_sb, w, b_s, dst, o, wid):
                    sl = slice(o, o + wid)
                    ps = psP.tile([E, 512], F32, tag="pp",
                                  name=f"p_{dst.tensor.name}_{o}")
                    for j in range(DC):
                        nc.tensor.matmul(ps[:, 0:wid], wslice(w, j),
                                         src_sb[:, j, sl],
                                         start=(j == 0), stop=(j == DC - 1))
                    nc.vector.tensor_scalar_add(dst[0:E, sl], ps[:, 0:wid],
                                                b_s)
                    nc.vector.tensor_copy(out=dst[E:2 * E, sl],
                                          in_=dst[0:E, sl])

                def proj_v(c0, nch):
                    ps = psV.tile([P, 4, E], F32, tag="pv", name=f"v_{c0}")
                    for ci in range(nch):
                        c = c0 + ci
                        for j in range(DC):
                            nc.tensor.matmul(
                                ps[:, ci, :], vT_sb[:, j, c * P:(c + 1) * P],
                                wslice(2, j), start=(j == 0),
                                stop=(j == DC - 1))
                    nc.vector.tensor_copy(
                        out=V1[:, c0:c0 + nch, 0:E], in_=ps[:, 0:nch, :])

                # k/v head of the pipeline first: it is ready before qT
                proj_qk(kT_sb, 1, bk_s, KT, 0, 512)
                proj_v(0, 4)
                proj_qk(qT_sb, 0, bq_s, QT, 0, 512)
                proj_qk(qT_sb, 0, bq_s, QT, 512, 512)

                for h in (0, 1):
                    pv_tiles[h] = psPV.tile([E + 1, HW], F32,
                                            tag=f"pv{h % 2}", name=f"pv_{h}")
                for sub in range((t2 + 511) // 512):
                    o = sub * 512
                    wid = min(512, t2 - o)
                    if sub > 0:
                        proj_qk(kT_sb, 1, bk_s, KT, o, wid)
                        proj_v(o // P, wid // P)
                    for cp in range(wid // 256):
                        for h in (0, 1):
                            scores_exp(psS1, o // P + cp * 2, 2, h, 2)

            # ---------------- stage 2: remaining halves + epilogues ------
            # the deferred query projections (tb 2/3) run here in the PE
            # slack between score groups, off stage 1's critical path
            with (
                tc.tile_pool(name="psS2", bufs=2, space="PSUM") as psS2,
                tc.tile_pool(name="psO", bufs=1, space="PSUM") as psO,
            ):
                def proj_q2(tb):
                    sl = slice(tb * 512, (tb + 1) * 512)
                    ps = psO.tile([E, 512], F32, tag="o0", name=f"q2_{tb}")
                    for j in range(DC):
                        nc.tensor.matmul(ps, wslice(0, j), qT_sb[:, j, sl],
                                         start=(j == 0), stop=(j == DC - 1))
                    nc.vector.tensor_scalar_add(QT[0:E, sl], ps, bq_s)
                    nc.vector.tensor_copy(out=QT[E:2 * E, sl],
                                          in_=QT[0:E, sl])

                flush_pv()
                stream_epi = [0, 1]
                proj_q2(2)
                epilogue(stream_epi.pop(0), psO)
                for h in (2, 3):
                    pv_tiles[h] = psPV.tile([E + 1, HW], F32,
                                            tag=f"pv{h % 2}", name=f"pv_{h}")
                    ng = NT2 // 2
                    for g in range(ng):
                        scores_exp(psS2, g * 2, 2, h, 2)
                        if h == 2 and g == 2 and stream_epi:
                            epilogue(stream_epi.pop(0), psO)
                        if h == 2 and g == 5:
                            proj_q2(3)
                        if h == 3 and g == ng - 2:
                            # drain PV early so only the last group's PV
                            # remains after the final exp
                            flush_pv()
                    flush_pv()
                    epilogue(h, psO)


def build_nc(t1l=T1 // 2, t2=T2):
    nc = bacc.Bacc()
    qT = nc.declare_dram_parameter("qT", [D, t1l], BF16, isOutput=False)
    kT = nc.declare_dram_parameter("kT", [D, t2], BF16, isOutput=False)
    vT = nc.declare_dram_parameter("vT", [D, t2], BF16, isOutput=False)
    cpkf = nc.declare_dram_parameter("cpkf", [P * 98], F32, isOutput=False)
    cpkb = nc.declare_dram_parameter("cpkb", [3 * D * E], BF16,
                                     isOutput=False)
    out = nc.declare_dram_parameter("out", [t1l, E], F32, isOutput=True)
    with tile.TileContext(nc) as tc:
        build_body(tc, nc, qT[:], kT[:], vT[:], cpkf[:], cpkb[:], out[:],
                   t1l, t2)
    nc.compile()
    return nc


_NC_CACHE = {}


def _get_nc(t2):
    if t2 not in _NC_CACHE:
        _NC_CACHE[t2] = build_nc(t2=t2)
    return _NC_CACHE[t2]


def make_in_maps(q, k, v, mask, Wq, bq, Wk, bk, Wv, bv):
    t1l = T1 // 2
    q = np.asarray(q, np.float32)
    k = np.asarray(k, np.float32)
    v = np.asarray(v, np.float32)
    mask = np.asarray(mask, np.float32)
    qbf = q.astype(BF)

    # compact away masked keys (exact; see module docstring)
    valid = [np.nonzero(mask[b, 0] != 0.0)[0] for b in range(B)]
    if max(len(ix) for ix in valid) <= T2C:
        t2 = T2C
    else:
        t2 = T2
        valid = [np.arange(T2) for _ in range(B)]

    # packed bf16 constants, partition-major: row p = [w, chunk, e] with
    # value W_w[chunk*128 + p, e]
    ws = np.stack([np.asarray(W, np.float32).astype(BF)
                   for W in (Wq, Wk, Wv)])          # [3, 512, 64]
    cpkb = np.ascontiguousarray(
        ws.reshape(3, 4, P, E).transpose(2, 0, 1, 3).reshape(P, -1)
    ).reshape(-1)

    in_maps = []
    for c in range(8):
        b, h = divmod(c, 2)
        ix = valid[b]
        nv = len(ix)
        kc = np.zeros((t2, D), BF)
        kc[:nv] = k[b, ix].astype(BF)
        vc = np.zeros((t2, D), BF)
        vc[:nv] = (v[b, ix] * mask[b, 0, ix, None]).astype(BF)
        mc = np.zeros(t2, np.float32)
        mc[:nv] = mask[b, 0, ix]
        # packed fp32 constants: mask cols | bq | bk | bv broadcast
        cpkf = np.zeros((P, 98), np.float32)
        cpkf[:, 0:t2 // P] = mc.reshape(t2 // P, P).T
        cpkf[0:E, 32] = np.asarray(bq, np.float32)
        cpkf[0:E, 33] = np.asarray(bk, np.float32)
        cpkf[:, 34:98] = np.asarray(bv, np.float32)[None, :]
        in_maps.append({
            "qT": np.ascontiguousarray(qbf[b, h * t1l:(h + 1) * t1l].T),
            "kT": np.ascontiguousarray(kc.T),
            "vT": np.ascontiguousarray(vc.T),
            "cpkf": np.ascontiguousarray(cpkf.reshape(-1)),
            "cpkb": cpkb,
        })
    return in_maps, t2


def assemble_out(results):
    t1l = T1 // 2
    out = np.empty((B, T1, E), np.float32)
    for c in range(8):
        b, h = divmod(c, 2)
        out[b, h * t1l:(h + 1) * t1l] = results[c]["out"]
    return out


def run(inputs, trace=False):
    from concourse.bass_utils import run_bass_kernel_spmd
    _install_ntff_hook()
    in_maps, t2 = make_in_maps(**inputs)
    nc = _get_nc(t2)
    res = run_bass_kernel_spmd(nc, in_maps, list(range(8)), trace=trace)
    return assemble_out(res.results), res


def kernel(q, k, v, mask, Wq, bq, Wk, bk, Wv, bv):
    out, _ = run(dict(q=q, k=k, v=v, mask=mask, Wq=Wq, bq=bq, Wk=Wk, bk=bk,
                      Wv=Wv, bv=bv))
    return out
